# revision 5
# baseline (speedup 1.0000x reference)
"""Trainium2 Bass kernel for nn_NeuralNet_27943057228550 (dense_mlp).

4-layer MLP with per-feature mixed activations + dropout masks + log_softmax.
Data-parallel over 8 NeuronCores (batch sharded); weights replicated.

Device dataflow is feature-major ("transposed"): activations live as
aT [feature, batch] tiles so each layer's matmul consumes the previous
layer's output directly and per-feature bias / activation segmentation is
per-partition (ACT bias APs are free). Host pre-permutes features of each
hidden layer so same-activation features are contiguous; engine
partition-range instructions must start 32-aligned, so segments are
emitted in reverse order with starts aligned down (over-covered rows are
overwritten by the correct earlier segment). x and the dropout masks are
sent pre-transposed (and mask columns pre-permuted) from the host.

Matmuls run as float32r (full PE rate, ~tf32 precision). exp() uses the
tanh identity e^x=(1+tanh(x/2))/(1-tanh(x/2)) so every hidden-layer ACT
function lives in one activation table set; the final log_softmax's ln()
runs once at the end (a single table switch).
"""
import contextlib
import math

import numpy as np

import concourse.bacc as bacc
import concourse.bass as bass
import concourse.tile as tile
import concourse.mybir as mybir
from concourse.bass_utils import run_bass_kernel_spmd

F32 = mybir.dt.float32
F32R = mybir.dt.float32r
AF = mybir.ActivationFunctionType
ALU = mybir.AluOpType

N_CORES = 8
B = 32768
D_IN = 784
H = 1024
C = 10
BL = B // N_CORES          # 4096 batch rows per core
BC = 512                   # batch chunk (matmul free dim)
KH = H // 128              # 8 hidden k-tiles
K0_FULL = D_IN // 128      # 6 full k-tiles for x
K0_REM = D_IN - K0_FULL * 128  # 16
MCH = H // 128             # 8 output-feature chunks per hidden layer
INV_SQRT2 = float(1.0 / math.sqrt(2.0))

# tid: 0 relu, 1 tanh, 2 sig, 3 smin, 4 smax, 5 gelu, 6 lrelu
ORDER = [1, 2, 5, 3, 4, 0, 6]


def split_prange(r0, r1):
    """Engine partition ranges must start at 0/32/64/96 and not cross an
    aligned block from a nonzero start. Decompose [r0, r1) accordingly
    (r0 must already be 32-aligned)."""
    if r0 == 0 or r0 >= 64 or r1 <= 64:
        return [(r0, r1)]
    return [(r0, 64), (64, r1)]


def _plan_layer(tids):
    rank = np.array([ORDER.index(t) for t in tids])
    perm = np.argsort(rank, kind="stable")
    sorted_t = np.array(tids)[perm]
    groups = []
    s = 0
    for i in range(1, H + 1):
        if i == H or sorted_t[i] != sorted_t[s]:
            groups.append((int(sorted_t[s]), s, i))
            s = i
    segs = []
    for (t, gs, ge) in groups:
        for m in range(gs // 128, (ge - 1) // 128 + 1):
            r0 = max(gs - m * 128, 0)
            r1 = min(ge - m * 128, 128)
            segs.append((m, r0, r1, t))
    sm_groups = {t: (gs, ge) for (t, gs, ge) in groups if t in (3, 4)}
    return perm, segs, sm_groups


class _Plan:
    pass


def _host_prep(x, W0, b0, W1, b1, W2, b2, W3, b3, mask1, mask2, mask3,
               type_ids):
    p = _Plan()
    perms, segss, smgs = [], [], []
    for l in range(3):
        perm, segs, smg = _plan_layer(type_ids[l])
        perms.append(perm); segss.append(segs); smgs.append(smg)
    p.perms, p.segss, p.smgs = perms, segss, smgs

    f32 = np.float32
    W0p = W0[perms[0]]
    W1p = (2.0 * W1[perms[1]][:, perms[0]]).astype(f32)
    W2p = (2.0 * W2[perms[2]][:, perms[1]]).astype(f32)
    W3p = (2.0 * W3[:, perms[2]]).astype(f32)
    b0p, b1p, b2p = b0[perms[0]], b1[perms[1]], b2[perms[2]]
    for l, Wn in ((0, W1p), (1, W2p), (2, W3p)):
        sorted_t = np.array(type_ids[l])[perms[l]]
        Wn[:, np.where(sorted_t == 5)[0]] *= 0.5
    p.w0t = np.ascontiguousarray(W0p.T, f32)             # [784, 1024]
    p.w1t = np.ascontiguousarray(W1p.T, f32)
    p.w2t = np.ascontiguousarray(W2p.T, f32)
    # w3 packed as one [128, 8*C] tile: k-tile k at cols [k*C:(k+1)*C]
    w3t = np.ascontiguousarray(W3p.T, f32)               # [1024, 10]
    p.w3pack = np.concatenate(
        [w3t[k * 128:(k + 1) * 128, :] for k in range(KH)], axis=1)

    bp = np.zeros((128, 96), f32)
    for l, bl_ in enumerate((b0p, b1p, b2p)):
        bm = bl_.reshape(MCH, 128).T
        bp[:, l * 32 + 0:l * 32 + 8] = bm
        bp[:, l * 32 + 8:l * 32 + 16] = 0.5 * bm
        bp[:, l * 32 + 16:l * 32 + 24] = -0.5 * bm
        bp[:, l * 32 + 24:l * 32 + 32] = INV_SQRT2 * bm
    p.bpack = bp
    b3p = np.zeros((16, 2), f32)
    b3p[:C, 0] = b3
    b3p[:C, 1] = 0.5 * b3
    p.b3pack = b3p

    # group-sum ones columns: (l, g, m) nonempty + ones10
    gcols, gindex = [], {}
    for l in range(3):
        for g in (3, 4):
            if g not in smgs[l]:
                continue
            gs, ge = smgs[l][g]
            for m in range(gs // 128, (ge - 1) // 128 + 1):
                r0, r1 = max(gs - m * 128, 0), min(ge - m * 128, 128)
                col = np.zeros(128, f32)
                col[r0:r1] = 1.0
                gindex[(l, g, m)] = len(gcols)
                gcols.append(col)
    ones10 = np.zeros(128, f32); ones10[:C] = 1.0
    gindex["ones10"] = len(gcols); gcols.append(ones10)
    p.gsum = np.ascontiguousarray(np.stack(gcols, axis=1))
    p.gindex = gindex

    # broadcast rows: per (l,g,m) the group row; plus an all-ones row.
    # bc = onesrow x ones512 (accum base 1.0) + grouprow x (recipS - 1)
    rrows, rindex = [], {}
    for key, ci in list(gindex.items()):
        if not isinstance(key, tuple):
            continue
        rindex[key] = len(rrows)
        rrows.append(p.gsum[:, ci].copy())
    rindex["ones"] = len(rrows); rrows.append(np.ones(128, f32))
    p.grow = np.ascontiguousarray(np.concatenate(rrows)[None, :])
    p.rindex = rindex
    p.ones512 = np.ones((1, BC), f32)

    p.xT = np.ascontiguousarray(x.T, f32)                # [784, B]
    p.mT = [np.ascontiguousarray(mask1.T[perms[0]], f32),
            np.ascontiguousarray(mask2.T[perms[1]], f32),
            np.ascontiguousarray(mask3.T[perms[2]], f32)]
    return p


def build_program(p, n_chunks):
    bl = n_chunks * BC
    nc = bacc.Bacc("TRN2", target_bir_lowering=False, debug=False)

    d_x = nc.dram_tensor("xT", [D_IN, bl], F32R, kind="ExternalInput")
    d_m = [nc.dram_tensor(f"m{l+1}T", [H, bl], F32R, kind="ExternalInput")
           for l in range(3)]
    d_w = [nc.dram_tensor("w0t", [D_IN, H], F32R, kind="ExternalInput"),
           nc.dram_tensor("w1t", [H, H], F32R, kind="ExternalInput"),
           nc.dram_tensor("w2t", [H, H], F32R, kind="ExternalInput"),
           nc.dram_tensor("w3pack", [128, KH * C], F32R, kind="ExternalInput")]
    d_bp = nc.dram_tensor("bpack", [128, 96], F32, kind="ExternalInput")
    d_b3 = nc.dram_tensor("b3pack", [16, 2], F32, kind="ExternalInput")
    nsum = p.gsum.shape[1]
    d_gs = nc.dram_tensor("gsum", [128, nsum], F32R, kind="ExternalInput")
    nrow = p.grow.shape[1]
    d_gr = nc.dram_tensor("grow", [1, nrow], F32R, kind="ExternalInput")
    d_o512 = nc.dram_tensor("ones512", [1, BC], F32R, kind="ExternalInput")
    d_zs = nc.dram_tensor("zscratch", [C, bl], F32, kind="Internal")
    d_out = nc.dram_tensor("outT", [C, bl], F32, kind="ExternalOutput")

    with tile.TileContext(nc) as tc:
        with contextlib.ExitStack() as ctx:
            const = ctx.enter_context(tc.tile_pool(name="const", bufs=1))
            wpool = ctx.enter_context(tc.tile_pool(name="w", bufs=1))
            apool = ctx.enter_context(tc.tile_pool(name="a", bufs=1))
            hpool = ctx.enter_context(tc.tile_pool(name="h", bufs=3))
            mpool = ctx.enter_context(tc.tile_pool(name="m", bufs=3))
            spool = ctx.enter_context(tc.tile_pool(name="scr", bufs=2))
            fpool = ctx.enter_context(tc.tile_pool(name="fin", bufs=1))
            ps = ctx.enter_context(tc.tile_pool(name="ps", bufs=3, space="PSUM"))
            pss = ctx.enter_context(tc.tile_pool(name="pss", bufs=1, space="PSUM"))
            psb = ctx.enter_context(tc.tile_pool(name="psb", bufs=2, space="PSUM"))

            bp_t = const.tile([128, 96], F32)
            nc.sync.dma_start(bp_t[:], d_bp.ap())
            b3_t = const.tile([16, 2], F32)
            nc.sync.dma_start(b3_t[:], d_b3.ap())
            gs_t = const.tile([128, nsum], F32R)
            nc.sync.dma_start(gs_t[:], d_gs.ap())
            gr_t = const.tile([1, nrow], F32R)
            nc.sync.dma_start(gr_t[:], d_gr.ap())
            o512_t = const.tile([1, BC], F32R)
            nc.sync.dma_start(o512_t[:], d_o512.ap())

            w0k = []
            for k in range(K0_FULL):
                t = wpool.tile([128, H], F32R, tag=f"w0_{k}")
                nc.sync.dma_start(t[:], d_w[0].ap()[k * 128:(k + 1) * 128, :])
                w0k.append(t)
            t = wpool.tile([K0_REM, H], F32R, tag="w0_r")
            nc.sync.dma_start(t[:], d_w[0].ap()[K0_FULL * 128:D_IN, :])
            w0k.append(t)
            wk = {0: w0k}
            for l in (1, 2):
                tiles = []
                for k in range(KH):
                    t = wpool.tile([128, H], F32R, tag=f"w{l}_{k}")
                    nc.sync.dma_start(t[:], d_w[l].ap()[k * 128:(k + 1) * 128, :])
                    tiles.append(t)
                wk[l] = tiles
            w3_t = wpool.tile([128, KH * C], F32R, tag="w3")
            nc.sync.dma_start(w3_t[:], d_w[3].ap())

            sfull = fpool.tile([1, bl], F32R)

            bias_ap = lambda l, kind, m, r0, r1: bp_t[r0:r1,
                l * 32 + kind * 8 + m:l * 32 + kind * 8 + m + 1]

            for cix in range(n_chunks):
                cs = cix * BC

                # L0 input k-tiles straight from DRAM (host-transposed x)
                a_cur = []
                for k in range(K0_FULL + 1):
                    kw = 128 if k < K0_FULL else K0_REM
                    st = apool.tile([kw, BC], F32R, tag=f"ae{k}",
                                    name=f"a0_{k}")
                    nc.sync.dma_start(
                        st[:], d_x.ap()[k * 128:k * 128 + kw, cs:cs + BC])
                    a_cur.append(st)

                for l in range(3):
                    nk = len(a_cur)
                    a_next = []
                    psum_s = {}
                    sm_ms = {}
                    for g in (3, 4):
                        ms = [m for m in range(MCH) if (l, g, m) in p.gindex]
                        if ms:
                            sm_ms[g] = ms
                            psum_s[g] = pss.tile([1, BC], F32, tag=f"s{g}",
                                                 name=f"psum_s{g}")

                    for m in range(MCH):
                        pz = ps.tile([128, BC], F32, tag="main", name="pz")
                        for k in range(nk):
                            nc.tensor.matmul(
                                pz[:], wk[l][k][:, m * 128:(m + 1) * 128],
                                a_cur[k][:], start=(k == 0),
                                stop=(k == nk - 1))
                        ht = hpool.tile([128, BC], F32R, tag="ht", name="ht")
                        msegs = [s for s in p.segss[l] if s[0] == m]
                        msegs.sort(key=lambda s: -s[1])
                        for (_, r0, r1s, t_) in msegs:
                          for (r0a, r1) in split_prange(r0 - (r0 % 32), r1s):
                            if t_ == 1:
                                nc.scalar.activation(
                                    ht[r0a:r1, :], pz[r0a:r1, :], AF.Tanh,
                                    bias=bias_ap(l, 0, m, r0a, r1))
                            elif t_ == 2:
                                nc.scalar.activation(
                                    ht[r0a:r1, :], pz[r0a:r1, :], AF.Sigmoid,
                                    bias=bias_ap(l, 0, m, r0a, r1))
                            elif t_ == 0:
                                nc.scalar.activation(
                                    ht[r0a:r1, :], pz[r0a:r1, :], AF.Relu,
                                    bias=bias_ap(l, 0, m, r0a, r1))
                            elif t_ == 6:
                                sc = spool.tile([128, BC], F32, tag="s1",
                                                name="sc_lr")
                                nc.vector.tensor_scalar(
                                    sc[r0a:r1, :], pz[r0a:r1, :],
                                    bias_ap(l, 0, m, r0a, r1), 0.01,
                                    op0=ALU.add, op1=ALU.mult)
                                nc.vector.scalar_tensor_tensor(
                                    ht[r0a:r1, :], pz[r0a:r1, :],
                                    bias_ap(l, 0, m, r0a, r1), sc[r0a:r1, :],
                                    op0=ALU.add, op1=ALU.max)
                            elif t_ == 5:
                                nc.scalar.activation(
                                    ht[r0a:r1, :], pz[r0a:r1, :], AF.Identity,
                                    bias=bias_ap(l, 0, m, r0a, r1))
                                sc = spool.tile([128, BC], F32, tag="s1",
                                                name="sc_ge")
                                nc.scalar.activation(
                                    sc[r0a:r1, :], pz[r0a:r1, :], AF.Erf,
                                    bias=bias_ap(l, 3, m, r0a, r1),
                                    scale=INV_SQRT2)
                                nc.vector.scalar_tensor_tensor(
                                    ht[r0a:r1, :], sc[r0a:r1, :], 1.0,
                                    ht[r0a:r1, :], op0=ALU.add, op1=ALU.mult)
                            else:
                                kind = 1 if t_ == 3 else 2
                                sgn = 0.5 if t_ == 3 else -0.5
                                tt = spool.tile([128, BC], F32, tag="s1",
                                                name="tt_sm")
                                nc.scalar.activation(
                                    tt[r0a:r1, :], pz[r0a:r1, :], AF.Tanh,
                                    bias=bias_ap(l, kind, m, r0a, r1),
                                    scale=sgn)
                                dd = spool.tile([128, BC], F32, tag="s2",
                                                name="dd_sm")
                                nc.vector.tensor_scalar(
                                    dd[r0a:r1, :], tt[r0a:r1, :], -1.0, 1.0,
                                    op0=ALU.mult, op1=ALU.add)
                                rr = spool.tile([128, BC], F32, tag="s3",
                                                name="rr_sm")
                                nc.vector.reciprocal(rr[r0a:r1, :],
                                                     dd[r0a:r1, :])
                                nc.vector.scalar_tensor_tensor(
                                    ht[r0a:r1, :], tt[r0a:r1, :], 1.0,
                                    rr[r0a:r1, :], op0=ALU.add, op1=ALU.mult)
                        for g in psum_s:
                            if (l, g, m) not in p.gindex:
                                continue
                            ci = p.gindex[(l, g, m)]
                            nc.tensor.matmul(
                                psum_s[g][:], gs_t[:, ci:ci + 1], ht[:],
                                start=(m == sm_ms[g][0]),
                                stop=(m == sm_ms[g][-1]),
                                skip_group_check=True)

                        mt = mpool.tile([128, BC], F32R, tag="mt", name="mt")
                        nc.sync.dma_start(
                            mt[:], d_m[l].ap()[m * 128:(m + 1) * 128,
                                               cs:cs + BC])
                        at = apool.tile([128, BC], F32R,
                                        tag=f"a{'o' if l % 2 == 0 else 'e'}{m}",
                                        name=f"a{l+1}_{m}")
                        nc.vector.tensor_tensor(at[:], ht[:], mt[:],
                                                op=ALU.mult)
                        a_next.append(at)

                    ri_ones = p.rindex["ones"] * 128
                    for g in psum_s:
                        rs = spool.tile([1, BC], F32R, tag="rs", name="rs")
                        with nc.allow_low_precision(reason="softmax recip"):
                            nc.vector.reciprocal(rs[:], psum_s[g][:])
                        rm1 = spool.tile([1, BC], F32R, tag="rs2", name="rm1")
                        with nc.allow_low_precision(reason="softmax recip"):
                            nc.vector.tensor_scalar(
                                rm1[:], rs[:], -1.0, None, op0=ALU.add)
                        for m in sm_ms[g]:
                            ri = p.rindex[(l, g, m)] * 128
                            pb = psb.tile([128, BC], F32, tag="bc", name="pb")
                            nc.tensor.matmul(
                                pb[:], gr_t[:, ri_ones:ri_ones + 128],
                                o512_t[:], start=True, stop=False,
                                skip_group_check=True)
                            nc.tensor.matmul(
                                pb[:], gr_t[:, ri:ri + 128], rm1[:],
                                start=False, stop=True, skip_group_check=True)
                            col = p.gsum[:, p.gindex[(l, g, m)]]
                            rows = np.where(col > 0)[0]
                            r0, r1 = int(rows[0]), int(rows[-1]) + 1
                            for (ra, rb) in split_prange(r0 - (r0 % 32), r1):
                                nc.vector.tensor_tensor(
                                    a_next[m][ra:rb, :], a_next[m][ra:rb, :],
                                    pb[ra:rb, :], op=ALU.mult)
                    a_cur = a_next

                # final layer
                pl = ps.tile([C, BC], F32, tag="main", name="pl")
                for k in range(KH):
                    nc.tensor.matmul(pl[:], w3_t[:, k * C:(k + 1) * C],
                                     a_cur[k][:], start=(k == 0),
                                     stop=(k == KH - 1))
                zc = spool.tile([C, BC], F32, tag="s1", name="zc")
                nc.scalar.activation(zc[:], pl[:], AF.Identity,
                                     bias=b3_t[0:C, 0:1])
                nc.sync.dma_start(d_zs.ap()[:, cs:cs + BC], zc[:])
                tt = spool.tile([C, BC], F32, tag="s2", name="tt_f")
                nc.scalar.activation(tt[:], pl[:], AF.Tanh,
                                     bias=b3_t[0:C, 1:2], scale=0.5)
                dd = spool.tile([C, BC], F32, tag="s3", name="dd_f")
                nc.vector.tensor_scalar(dd[:], tt[:], -1.0, 1.0,
                                        op0=ALU.mult, op1=ALU.add)
                rr = spool.tile([C, BC], F32, tag="s1", name="rr_f")
                nc.vector.reciprocal(rr[:], dd[:])
                ee = spool.tile([C, BC], F32R, tag="s2", name="ee_f")
                with nc.allow_low_precision(reason="softmax exp"):
                    nc.vector.scalar_tensor_tensor(
                        ee[:], tt[:], 1.0, rr[:], op0=ALU.add, op1=ALU.mult)
                ci = p.gindex["ones10"]
                psl = pss.tile([1, BC], F32, tag="s3", name="psl")
                nc.tensor.matmul(psl[:], gs_t[0:C, ci:ci + 1], ee[:],
                                 start=True, stop=True, skip_group_check=True)
                nc.vector.tensor_copy(sfull[:, cs:cs + BC], psl[:])

            # finalize
            nc.scalar.activation(sfull[:], sfull[:], AF.Ln)
            ri = p.rindex["ones"] * 128
            for cix in range(n_chunks):
                cs = cix * BC
                pb = psb.tile([C, BC], F32, tag="bc", name="pb_f")
                nc.tensor.matmul(pb[:], gr_t[:, ri:ri + C],
                                 sfull[:, cs:cs + BC], start=True, stop=True,
                                 skip_group_check=True)
                zr = spool.tile([C, BC], F32, tag="s1", name="zr")
                nc.sync.dma_start(zr[:], d_zs.ap()[:, cs:cs + BC])
                oc = spool.tile([C, BC], F32, tag="s2", name="oc")
                nc.vector.tensor_tensor(oc[:], zr[:], pb[:], op=ALU.subtract)
                nc.sync.dma_start(d_out.ap()[:, cs:cs + BC], oc[:])

    nc.compile()
    return nc


def make_in_maps(p, n_chunks=None):
    bl = (n_chunks or (BL // BC)) * BC
    ncores = p.xT.shape[1] // bl
    shared = dict(w0t=p.w0t, w1t=p.w1t, w2t=p.w2t, w3pack=p.w3pack,
                  bpack=p.bpack, b3pack=p.b3pack, gsum=p.gsum,
                  grow=p.grow, ones512=p.ones512)
    maps = []
    for c in range(ncores):
        m = dict(shared)
        m["xT"] = p.xT[:, c * bl:(c + 1) * bl]
        for l in range(3):
            m[f"m{l+1}T"] = p.mT[l][:, c * bl:(c + 1) * bl]
        maps.append(m)
    return maps


_CACHE = {}


def kernel(x, W0, b0, W1, b1, W2, b2, W3, b3, mask1, mask2, mask3, type_ids):
    p = _host_prep(x, W0, b0, W1, b1, W2, b2, W3, b3,
                   mask1, mask2, mask3, type_ids)
    n_chunks = BL // BC
    key = tuple(np.asarray(type_ids).ravel().tolist())
    if key not in _CACHE:
        _CACHE[key] = build_program(p, n_chunks)
    nc = _CACHE[key]
    maps = make_in_maps(p, n_chunks)
    res = run_bass_kernel_spmd(nc, maps, core_ids=list(range(N_CORES)))
    out = np.concatenate([r["outT"].T for r in res.results], axis=0)
    return (np.ascontiguousarray(out, np.float32), mask1, mask2, mask3)


# revision 11
# speedup vs baseline: 18448.1228x; 18448.1228x over previous
"""Trainium2 Bass kernel for nn_NeuralNet_27943057228550 (dense_mlp).

4-layer MLP with per-feature mixed activations + dropout masks + log_softmax.
Data-parallel over 8 NeuronCores (batch sharded); weights replicated.

Device dataflow is feature-major ("transposed"): activations live as
aT [feature, batch] tiles so each layer's matmul consumes the previous
layer's output directly and per-feature bias / activation segmentation is
per-partition (ACT bias APs are free). Host pre-permutes features of each
hidden layer so same-activation features are contiguous; engine
partition-range instructions must start 32-aligned, so segments are
emitted in reverse order with starts aligned down (over-covered rows are
overwritten by the correct earlier segment). x and the dropout masks are
sent pre-transposed (and mask columns pre-permuted) from the host.

Matmuls run as float32r (full PE rate, ~tf32 precision). exp() uses the
tanh identity e^x=(1+tanh(x/2))/(1-tanh(x/2)) so every hidden-layer ACT
function lives in one activation table set; the final log_softmax's ln()
runs once at the end (a single table switch).
"""
import contextlib
import math

import numpy as np

import concourse.bacc as bacc
import concourse.bass as bass
import concourse.tile as tile
import concourse.mybir as mybir
from concourse.bass_utils import run_bass_kernel_spmd

F32 = mybir.dt.float32
F32R = mybir.dt.float32r
AF = mybir.ActivationFunctionType
ALU = mybir.AluOpType

N_CORES = 8
B = 32768
D_IN = 784
H = 1024
C = 10
BL = B // N_CORES          # 4096 batch rows per core
BC = 512                   # batch chunk (matmul free dim)
KH = H // 128              # 8 hidden k-tiles
K0_FULL = D_IN // 128      # 6 full k-tiles for x
K0_REM = D_IN - K0_FULL * 128  # 16
MCH = H // 128             # 8 output-feature chunks per hidden layer
INV_SQRT2 = float(1.0 / math.sqrt(2.0))

# tid: 0 relu, 1 tanh, 2 sig, 3 smin, 4 smax, 5 gelu, 6 lrelu
ORDER = [1, 2, 5, 3, 4, 0, 6]


def split_prange(r0, r1):
    """Engine partition ranges must start at 0/32/64/96 and not cross an
    aligned block from a nonzero start. Decompose [r0, r1) accordingly
    (r0 must already be 32-aligned)."""
    if r0 == 0 or r0 >= 64 or r1 <= 64:
        return [(r0, r1)]
    return [(r0, 64), (64, r1)]


def _plan_layer(tids):
    rank = np.array([ORDER.index(t) for t in tids])
    perm = np.argsort(rank, kind="stable")
    sorted_t = np.array(tids)[perm]
    groups = []
    s = 0
    for i in range(1, H + 1):
        if i == H or sorted_t[i] != sorted_t[s]:
            groups.append((int(sorted_t[s]), s, i))
            s = i
    segs = []
    for (t, gs, ge) in groups:
        for m in range(gs // 128, (ge - 1) // 128 + 1):
            r0 = max(gs - m * 128, 0)
            r1 = min(ge - m * 128, 128)
            segs.append((m, r0, r1, t))
    sm_groups = {t: (gs, ge) for (t, gs, ge) in groups if t in (3, 4)}
    return perm, segs, sm_groups


class _Plan:
    pass


def _host_prep(x, W0, b0, W1, b1, W2, b2, W3, b3, mask1, mask2, mask3,
               type_ids):
    p = _Plan()
    perms, segss, smgs = [], [], []
    for l in range(3):
        perm, segs, smg = _plan_layer(type_ids[l])
        perms.append(perm); segss.append(segs); smgs.append(smg)
    p.perms, p.segss, p.smgs = perms, segss, smgs

    f32 = np.float32
    W0p = W0[perms[0]]
    W1p = (2.0 * W1[perms[1]][:, perms[0]]).astype(f32)
    W2p = (2.0 * W2[perms[2]][:, perms[1]]).astype(f32)
    W3p = (2.0 * W3[:, perms[2]]).astype(f32)
    b0p, b1p, b2p = b0[perms[0]], b1[perms[1]], b2[perms[2]]
    for l, Wn in ((0, W1p), (1, W2p), (2, W3p)):
        sorted_t = np.array(type_ids[l])[perms[l]]
        Wn[:, np.where(sorted_t == 5)[0]] *= 0.5
    p.w0t = np.ascontiguousarray(W0p.T, f32)             # [784, 1024]
    p.w1t = np.ascontiguousarray(W1p.T, f32)
    p.w2t = np.ascontiguousarray(W2p.T, f32)
    # w3 packed as one [128, 8*C] tile: k-tile k at cols [k*C:(k+1)*C]
    w3t = np.ascontiguousarray(W3p.T, f32)               # [1024, 10]
    p.w3pack = np.concatenate(
        [w3t[k * 128:(k + 1) * 128, :] for k in range(KH)], axis=1)

    bp = np.zeros((128, 96), f32)
    for l, bl_ in enumerate((b0p, b1p, b2p)):
        bm = bl_.reshape(MCH, 128).T
        bp[:, l * 32 + 0:l * 32 + 8] = bm
        bp[:, l * 32 + 8:l * 32 + 16] = 0.5 * bm
        bp[:, l * 32 + 16:l * 32 + 24] = -0.5 * bm
        bp[:, l * 32 + 24:l * 32 + 32] = INV_SQRT2 * bm
    p.bpack = bp
    b3p = np.zeros((16, 2), f32)
    b3p[:C, 0] = b3
    b3p[:C, 1] = 0.5 * b3
    p.b3pack = b3p

    # group-sum ones columns: (l, g, m) nonempty + ones10
    gcols, gindex = [], {}
    for l in range(3):
        for g in (3, 4):
            if g not in smgs[l]:
                continue
            gs, ge = smgs[l][g]
            for m in range(gs // 128, (ge - 1) // 128 + 1):
                r0, r1 = max(gs - m * 128, 0), min(ge - m * 128, 128)
                col = np.zeros(128, f32)
                col[r0:r1] = 1.0
                gindex[(l, g, m)] = len(gcols)
                gcols.append(col)
    ones10 = np.zeros(128, f32); ones10[:C] = 1.0
    gindex["ones10"] = len(gcols); gcols.append(ones10)
    p.gsum = np.ascontiguousarray(np.stack(gcols, axis=1))
    p.gindex = gindex

    # broadcast rows: per (l,g,m) the group row; plus an all-ones row.
    # bc = onesrow x ones512 (accum base 1.0) + grouprow x (recipS - 1)
    rrows, rindex = [], {}
    for key, ci in list(gindex.items()):
        if not isinstance(key, tuple):
            continue
        rindex[key] = len(rrows)
        rrows.append(p.gsum[:, ci].copy())
    rindex["ones"] = len(rrows); rrows.append(np.ones(128, f32))
    p.grow = np.ascontiguousarray(np.concatenate(rrows)[None, :])
    p.rindex = rindex
    p.ones512 = np.ones((1, BC), f32)

    p.xT = np.ascontiguousarray(x.T, f32)                # [784, B]
    p.mT = [np.ascontiguousarray(mask1.T[perms[0]], f32),
            np.ascontiguousarray(mask2.T[perms[1]], f32),
            np.ascontiguousarray(mask3.T[perms[2]], f32)]
    return p


def build_program(p, n_chunks):
    bl = n_chunks * BC
    nc = bacc.Bacc("TRN2", target_bir_lowering=False, debug=False)

    d_x = nc.dram_tensor("xT", [D_IN, bl], F32R, kind="ExternalInput")
    d_m = [nc.dram_tensor(f"m{l+1}T", [H, bl], F32R, kind="ExternalInput")
           for l in range(3)]
    d_w = [nc.dram_tensor("w0t", [D_IN, H], F32R, kind="ExternalInput"),
           nc.dram_tensor("w1t", [H, H], F32R, kind="ExternalInput"),
           nc.dram_tensor("w2t", [H, H], F32R, kind="ExternalInput"),
           nc.dram_tensor("w3pack", [128, KH * C], F32R, kind="ExternalInput")]
    d_bp = nc.dram_tensor("bpack", [128, 96], F32, kind="ExternalInput")
    d_b3 = nc.dram_tensor("b3pack", [16, 2], F32, kind="ExternalInput")
    nsum = p.gsum.shape[1]
    d_gs = nc.dram_tensor("gsum", [128, nsum], F32R, kind="ExternalInput")
    nrow = p.grow.shape[1]
    d_gr = nc.dram_tensor("grow", [1, nrow], F32R, kind="ExternalInput")
    d_o512 = nc.dram_tensor("ones512", [1, BC], F32R, kind="ExternalInput")
    d_zs = nc.dram_tensor("zscratch", [C, bl], F32, kind="Internal")
    d_out = nc.dram_tensor("outT", [C, bl], F32, kind="ExternalOutput")

    with tile.TileContext(nc) as tc:
        with contextlib.ExitStack() as ctx:
            const = ctx.enter_context(tc.tile_pool(name="const", bufs=1))
            wpool = ctx.enter_context(tc.tile_pool(name="w", bufs=1))
            apool = ctx.enter_context(tc.tile_pool(name="a", bufs=1))
            hpool = ctx.enter_context(tc.tile_pool(name="h", bufs=3))
            mpool = ctx.enter_context(tc.tile_pool(name="m", bufs=3))
            spool = ctx.enter_context(tc.tile_pool(name="scr", bufs=2))
            fpool = ctx.enter_context(tc.tile_pool(name="fin", bufs=1))
            ps = ctx.enter_context(tc.tile_pool(name="ps", bufs=3, space="PSUM"))
            pss = ctx.enter_context(tc.tile_pool(name="pss", bufs=1, space="PSUM"))
            psb = ctx.enter_context(tc.tile_pool(name="psb", bufs=2, space="PSUM"))

            bp_t = const.tile([128, 96], F32)
            nc.sync.dma_start(bp_t[:], d_bp.ap())
            b3_t = const.tile([16, 2], F32)
            nc.sync.dma_start(b3_t[:], d_b3.ap())
            gs_t = const.tile([128, nsum], F32R)
            nc.sync.dma_start(gs_t[:], d_gs.ap())
            gr_t = const.tile([1, nrow], F32R)
            nc.sync.dma_start(gr_t[:], d_gr.ap())
            o512_t = const.tile([1, BC], F32R)
            nc.sync.dma_start(o512_t[:], d_o512.ap())

            w0k = []
            for k in range(K0_FULL):
                t = wpool.tile([128, H], F32R, tag=f"w0_{k}")
                nc.sync.dma_start(t[:], d_w[0].ap()[k * 128:(k + 1) * 128, :])
                w0k.append(t)
            t = wpool.tile([K0_REM, H], F32R, tag="w0_r")
            nc.sync.dma_start(t[:], d_w[0].ap()[K0_FULL * 128:D_IN, :])
            w0k.append(t)
            wk = {0: w0k}
            for l in (1, 2):
                tiles = []
                for k in range(KH):
                    t = wpool.tile([128, H], F32R, tag=f"w{l}_{k}")
                    nc.sync.dma_start(t[:], d_w[l].ap()[k * 128:(k + 1) * 128, :])
                    tiles.append(t)
                wk[l] = tiles
            w3_t = wpool.tile([128, KH * C], F32R, tag="w3")
            nc.sync.dma_start(w3_t[:], d_w[3].ap())

            sfull = fpool.tile([1, bl], F32R)

            bias_ap = lambda l, kind, m, r0, r1: bp_t[r0:r1,
                l * 32 + kind * 8 + m:l * 32 + kind * 8 + m + 1]

            for cix in range(n_chunks):
                cs = cix * BC

                # L0 input k-tiles straight from DRAM (host-transposed x)
                a_cur = []
                for k in range(K0_FULL + 1):
                    kw = 128 if k < K0_FULL else K0_REM
                    st = apool.tile([kw, BC], F32R, tag=f"ae{k}",
                                    name=f"a0_{k}")
                    nc.sync.dma_start(
                        st[:], d_x.ap()[k * 128:k * 128 + kw, cs:cs + BC])
                    a_cur.append(st)

                for l in range(3):
                    nk = len(a_cur)
                    a_next = []
                    psum_s = {}
                    sm_ms = {}
                    for g in (3, 4):
                        ms = [m for m in range(MCH) if (l, g, m) in p.gindex]
                        if ms:
                            sm_ms[g] = ms
                            psum_s[g] = pss.tile([1, BC], F32, tag=f"s{g}",
                                                 name=f"psum_s{g}")

                    for m in range(MCH):
                        pz = ps.tile([128, BC], F32, tag="main", name="pz")
                        for k in range(nk):
                            nc.tensor.matmul(
                                pz[:], wk[l][k][:, m * 128:(m + 1) * 128],
                                a_cur[k][:], start=(k == 0),
                                stop=(k == nk - 1))
                        ht = hpool.tile([128, BC], F32R, tag="ht", name="ht")
                        msegs = [s for s in p.segss[l] if s[0] == m]
                        msegs.sort(key=lambda s: -s[1])
                        for (_, r0, r1s, t_) in msegs:
                          for (r0a, r1) in split_prange(r0 - (r0 % 32), r1s):
                            if t_ == 1:
                                nc.scalar.activation(
                                    ht[r0a:r1, :], pz[r0a:r1, :], AF.Tanh,
                                    bias=bias_ap(l, 0, m, r0a, r1))
                            elif t_ == 2:
                                nc.scalar.activation(
                                    ht[r0a:r1, :], pz[r0a:r1, :], AF.Sigmoid,
                                    bias=bias_ap(l, 0, m, r0a, r1))
                            elif t_ == 0:
                                nc.scalar.activation(
                                    ht[r0a:r1, :], pz[r0a:r1, :], AF.Relu,
                                    bias=bias_ap(l, 0, m, r0a, r1))
                            elif t_ == 6:
                                sc = spool.tile([128, BC], F32, tag="s1",
                                                name="sc_lr")
                                nc.vector.tensor_scalar(
                                    sc[r0a:r1, :], pz[r0a:r1, :],
                                    bias_ap(l, 0, m, r0a, r1), 0.01,
                                    op0=ALU.add, op1=ALU.mult)
                                nc.vector.scalar_tensor_tensor(
                                    ht[r0a:r1, :], pz[r0a:r1, :],
                                    bias_ap(l, 0, m, r0a, r1), sc[r0a:r1, :],
                                    op0=ALU.add, op1=ALU.max)
                            elif t_ == 5:
                                nc.scalar.activation(
                                    ht[r0a:r1, :], pz[r0a:r1, :], AF.Identity,
                                    bias=bias_ap(l, 0, m, r0a, r1))
                                sc = spool.tile([128, BC], F32, tag="s1",
                                                name="sc_ge")
                                nc.scalar.activation(
                                    sc[r0a:r1, :], pz[r0a:r1, :], AF.Erf,
                                    bias=bias_ap(l, 3, m, r0a, r1),
                                    scale=INV_SQRT2)
                                nc.vector.scalar_tensor_tensor(
                                    ht[r0a:r1, :], sc[r0a:r1, :], 1.0,
                                    ht[r0a:r1, :], op0=ALU.add, op1=ALU.mult)
                            else:
                                kind = 1 if t_ == 3 else 2
                                sgn = 0.5 if t_ == 3 else -0.5
                                tt = spool.tile([128, BC], F32, tag="s1",
                                                name="tt_sm")
                                nc.scalar.activation(
                                    tt[r0a:r1, :], pz[r0a:r1, :], AF.Tanh,
                                    bias=bias_ap(l, kind, m, r0a, r1),
                                    scale=sgn)
                                dd = spool.tile([128, BC], F32, tag="s2",
                                                name="dd_sm")
                                nc.vector.tensor_scalar(
                                    dd[r0a:r1, :], tt[r0a:r1, :], -1.0, 1.0,
                                    op0=ALU.mult, op1=ALU.add)
                                rr = spool.tile([128, BC], F32, tag="s3",
                                                name="rr_sm")
                                nc.vector.reciprocal(rr[r0a:r1, :],
                                                     dd[r0a:r1, :])
                                nc.vector.scalar_tensor_tensor(
                                    ht[r0a:r1, :], tt[r0a:r1, :], 1.0,
                                    rr[r0a:r1, :], op0=ALU.add, op1=ALU.mult)
                        for g in psum_s:
                            if (l, g, m) not in p.gindex:
                                continue
                            ci = p.gindex[(l, g, m)]
                            nc.tensor.matmul(
                                psum_s[g][:], gs_t[:, ci:ci + 1], ht[:],
                                start=(m == sm_ms[g][0]),
                                stop=(m == sm_ms[g][-1]),
                                skip_group_check=True)

                        mt = mpool.tile([128, BC], F32R, tag="mt", name="mt")
                        nc.sync.dma_start(
                            mt[:], d_m[l].ap()[m * 128:(m + 1) * 128,
                                               cs:cs + BC])
                        at = apool.tile([128, BC], F32R,
                                        tag=f"a{'o' if l % 2 == 0 else 'e'}{m}",
                                        name=f"a{l+1}_{m}")
                        nc.vector.tensor_tensor(at[:], ht[:], mt[:],
                                                op=ALU.mult)
                        a_next.append(at)

                    ri_ones = p.rindex["ones"] * 128
                    for g in psum_s:
                        rs = spool.tile([1, BC], F32R, tag="rs", name="rs")
                        with nc.allow_low_precision(reason="softmax recip"):
                            nc.vector.reciprocal(rs[:], psum_s[g][:])
                        rm1 = spool.tile([1, BC], F32R, tag="rs2", name="rm1")
                        with nc.allow_low_precision(reason="softmax recip"):
                            nc.vector.tensor_scalar(
                                rm1[:], rs[:], -1.0, None, op0=ALU.add)
                        for m in sm_ms[g]:
                            ri = p.rindex[(l, g, m)] * 128
                            pb = psb.tile([128, BC], F32, tag="bc", name="pb")
                            nc.tensor.matmul(
                                pb[:], gr_t[:, ri_ones:ri_ones + 128],
                                o512_t[:], start=True, stop=False,
                                skip_group_check=True)
                            nc.tensor.matmul(
                                pb[:], gr_t[:, ri:ri + 128], rm1[:],
                                start=False, stop=True, skip_group_check=True)
                            col = p.gsum[:, p.gindex[(l, g, m)]]
                            rows = np.where(col > 0)[0]
                            r0, r1 = int(rows[0]), int(rows[-1]) + 1
                            for (ra, rb) in split_prange(r0 - (r0 % 32), r1):
                                nc.vector.tensor_tensor(
                                    a_next[m][ra:rb, :], a_next[m][ra:rb, :],
                                    pb[ra:rb, :], op=ALU.mult)
                    a_cur = a_next

                # final layer
                pl = ps.tile([C, BC], F32, tag="main", name="pl")
                for k in range(KH):
                    nc.tensor.matmul(pl[:], w3_t[:, k * C:(k + 1) * C],
                                     a_cur[k][:], start=(k == 0),
                                     stop=(k == KH - 1))
                zc = spool.tile([C, BC], F32, tag="s1", name="zc")
                nc.scalar.activation(zc[:], pl[:], AF.Identity,
                                     bias=b3_t[0:C, 0:1])
                nc.sync.dma_start(d_zs.ap()[:, cs:cs + BC], zc[:])
                tt = spool.tile([C, BC], F32, tag="s2", name="tt_f")
                nc.scalar.activation(tt[:], pl[:], AF.Tanh,
                                     bias=b3_t[0:C, 1:2], scale=0.5)
                dd = spool.tile([C, BC], F32, tag="s3", name="dd_f")
                nc.vector.tensor_scalar(dd[:], tt[:], -1.0, 1.0,
                                        op0=ALU.mult, op1=ALU.add)
                rr = spool.tile([C, BC], F32, tag="s1", name="rr_f")
                nc.vector.reciprocal(rr[:], dd[:])
                ee = spool.tile([C, BC], F32R, tag="s2", name="ee_f")
                with nc.allow_low_precision(reason="softmax exp"):
                    nc.vector.scalar_tensor_tensor(
                        ee[:], tt[:], 1.0, rr[:], op0=ALU.add, op1=ALU.mult)
                ci = p.gindex["ones10"]
                psl = pss.tile([1, BC], F32, tag="s3", name="psl")
                nc.tensor.matmul(psl[:], gs_t[0:C, ci:ci + 1], ee[:],
                                 start=True, stop=True, skip_group_check=True)
                nc.vector.tensor_copy(sfull[:, cs:cs + BC], psl[:])

            # finalize
            nc.scalar.activation(sfull[:], sfull[:], AF.Ln)
            ri = p.rindex["ones"] * 128
            for cix in range(n_chunks):
                cs = cix * BC
                pb = psb.tile([C, BC], F32, tag="bc", name="pb_f")
                nc.tensor.matmul(pb[:], gr_t[:, ri:ri + C],
                                 sfull[:, cs:cs + BC], start=True, stop=True,
                                 skip_group_check=True)
                zr = spool.tile([C, BC], F32, tag="s1", name="zr")
                nc.sync.dma_start(zr[:], d_zs.ap()[:, cs:cs + BC])
                oc = spool.tile([C, BC], F32, tag="s2", name="oc")
                nc.vector.tensor_tensor(oc[:], zr[:], pb[:], op=ALU.subtract)
                nc.sync.dma_start(d_out.ap()[:, cs:cs + BC], oc[:])

    nc.compile()
    return nc


def make_in_maps(p, n_chunks=None):
    bl = (n_chunks or (BL // BC)) * BC
    ncores = p.xT.shape[1] // bl
    shared = dict(w0t=p.w0t, w1t=p.w1t, w2t=p.w2t, w3pack=p.w3pack,
                  bpack=p.bpack, b3pack=p.b3pack, gsum=p.gsum,
                  grow=p.grow, ones512=p.ones512)
    maps = []
    for c in range(ncores):
        m = dict(shared)
        m["xT"] = p.xT[:, c * bl:(c + 1) * bl]
        for l in range(3):
            m[f"m{l+1}T"] = p.mT[l][:, c * bl:(c + 1) * bl]
        maps.append(m)
    return maps


_CACHE = {}


def kernel(x, W0, b0, W1, b1, W2, b2, W3, b3, mask1, mask2, mask3, type_ids):
    p = _host_prep(x, W0, b0, W1, b1, W2, b2, W3, b3,
                   mask1, mask2, mask3, type_ids)
    n_chunks = BL // BC
    key = tuple(np.asarray(type_ids).ravel().tolist())
    if key not in _CACHE:
        _CACHE[key] = build_program(p, n_chunks)
    nc = _CACHE[key]
    maps = make_in_maps(p, n_chunks)
    res = run_bass_kernel_spmd(nc, maps, core_ids=list(range(N_CORES)))
    out = np.concatenate([r["outT"].T for r in res.results], axis=0)
    return (np.ascontiguousarray(out, np.float32), mask1, mask2, mask3)
